# revision 1
# baseline (speedup 1.0000x reference)
"""Trainium2 Bass kernel for nn_CausalAttentionPooling.

Math: scores[b,i,j] = x[b,i].q are constant along the softmax axis j, so
softmax over the causal mask yields uniform weights 1/(i+1) on j <= i.
The module is exactly a causal cumulative mean:
    out[b,i,:] = cumsum(x, axis=1)[b,i,:] / (i+1)
(q does not affect the output.)

Sharding: 8 shards = (batch b in 0..3) x (D-half dh in 0..1); each core gets
x[b, :, dh*128:(dh+1)*128] transposed to [128(D), 4096(L)].

Measured HW facts driving the design:
  - aggregate DMA under profiling ~200 B/ns; out ships bf16 (host upcasts).
  - DVE scan: fp32-src ~2.3 ns/col, bf16-src ~3.3 ns/col -> x ships fp32.
  - SBUF->SBUF elementwise on Act/Pool concurrent with DVE slows the scan
    chain ~20-25% (shared SBUF bandwidth) -> nothing elementwise off-DVE.
  - all-bf16 tensor_tensor hits the DVE 2x_1p mode (~0.65 ns/col) -> cum
    and the finalize mults are bf16 (scan state itself stays fp32).
  - Pool elementwise work poisons DVE SBUF bandwidth -> Pool does nothing.
  - PE bf16 outer products replicate 1/(i+1) into PSUM; Act copies them
    to bf16 SBUF for the fins.
  - single-queue FIFO input in scan order; out-DMA issues alternate
    SP/Act so one engine's ~0.85us issue rate never paces the tail.
"""

import numpy as np

B, L, D = 4, 4096, 256
NCORES = 8
P = 128
PB = 512                 # psum bank width
NB = L // PB

_cache = {}


def _split_waits_bir(bir_bytes):
    """This container's walrus build rejects instructions carrying more than
    one (or for some opcodes, two) sync waits.  Hoist multi-wait sync_info
    onto standalone same-engine EventSemaphore instructions inserted
    immediately before the instruction; program order on the engine's stream
    preserves semantics."""
    import orjson

    d = orjson.loads(bir_bytes)
    n = 0
    for fn in d["functions"]:
        for bb in fn["blocks"]:
            out = []
            for inst in bb["instructions"]:
                si = inst.get("sync_info")
                waits = (si or {}).get("on_wait") or []
                if len(waits) > 1:
                    for w in waits:
                        out.append(
                            {
                                "debug": inst.get("debug"),
                                "engine": inst["engine"],
                                "ins": [],
                                "name": f"I-waitfix-{n}",
                                "opcode": "EventSemaphore",
                                "outs": [],
                                "sync_info": {"on_wait": [w], "on_update": []},
                            }
                        )
                        n += 1
                    si["on_wait"] = []
                out.append(inst)
            bb["instructions"] = out
    return orjson.dumps(d)


def _install_bir_patch():
    if _cache.get("patched"):
        return
    import concourse.bass as bass

    orig = bass.Bass.to_json_bytes

    def patched(self):
        return _split_waits_bir(orig(self))

    bass.Bass.to_json_bytes = patched
    _cache["patched"] = True


def _build_nc():
    import concourse.bass as bass
    import concourse.tile as tile
    from concourse import mybir

    _install_bir_patch()

    f32 = mybir.dt.float32
    bf16 = mybir.dt.bfloat16
    add = mybir.AluOpType.add
    byp = mybir.AluOpType.bypass
    mult = mybir.AluOpType.mult

    nc = bass.Bass()
    xT = nc.declare_dram_parameter("xT", [P, L], f32, isOutput=False)
    rrow = nc.declare_dram_parameter("rrow", [1, L], bf16, isOutput=False)
    out = nc.declare_dram_parameter("out", [P, L], bf16, isOutput=True)

    # scan spans (chained on DVE): ~400ns fixed cost per scan instruction
    # favors wide middles; small first/last spans for start latency / tail
    xspans = [(0, 128), (128, 640), (640, 1664), (1664, 2688),
              (2688, 3712), (3712, 3968), (3968, L)]

    with tile.TileContext(nc) as tc:
        with (
            tc.tile_pool(name="sb", bufs=1) as sb,
            tc.tile_pool(name="ps", bufs=1, space="PSUM") as ps,
        ):
            xt = sb.tile([P, L], f32, tag="xt")
            cum = sb.tile([P, L], bf16, tag="cum")
            ot = sb.tile([P, L], bf16, tag="ot")
            rrow_sb = sb.tile([1, L], bf16, tag="rrow")
            ones = sb.tile([1, P], bf16, tag="ones")
            rr_sb = sb.tile([P, L], bf16, tag="rrsb")

            # ---- input: rrow first (tiny, unblocks PE), then x spans in
            # scan order on the single SP queue (FIFO priority)
            nc.sync.dma_start(rrow_sb[:], rrow[:])
            for a, b in xspans:
                nc.sync.dma_start(xt[:, a:b], xT[:, a:b])

            # ---- ones memset on Pool (idle), PE replicates 1/(i+1)
            nc.gpsimd.memset(ones[:], 1.0)
            rr_ps = []
            for j in range(NB):
                pt = ps.tile([P, PB], f32, tag=f"rr{j}")
                nc.tensor.matmul(
                    pt[:],
                    ones[:],
                    rrow_sb[:, j * PB : (j + 1) * PB],
                    start=True,
                    stop=True,
                )
                rr_ps.append(pt)

            # ---- Act: copy scale banks PSUM->bf16 SBUF (PSUM-side reads
            # don't contend with the DVE's SBUF bandwidth; SBUF->SBUF
            # elementwise work on Act or Pool slows the scans ~20%)
            for c in range(NB):
                nc.scalar.copy(rr_sb[:, c * PB : (c + 1) * PB], rr_ps[c][:])

            # ---- DVE: chained scans (fp32 src, bf16 cum out; chaining
            # through the bf16 boundary column adds <=0.4% per hop) with
            # all-bf16 finalize mults trailing one span behind
            def scan(si):
                a, b = xspans[si]
                init = 0.0 if si == 0 else cum[:, a - 1 : a]
                nc.vector.tensor_tensor_scan(
                    cum[:, a:b], xt[:, a:b], xt[:, a:b], init, op0=add, op1=byp
                )

            def fin(si):
                a, b = xspans[si]
                nc.vector.tensor_tensor(
                    ot[:, a:b], cum[:, a:b], rr_sb[:, a:b], op=mult
                )
                eng = nc.sync if si % 2 == 0 else nc.scalar
                eng.dma_start(out[:, a:b], ot[:, a:b])

            nspans = len(xspans)
            for si in range(nspans):
                scan(si)
                if si >= 1:
                    fin(si - 1)
            fin(nspans - 1)
    return nc


def _get_nc():
    if "nc" not in _cache:
        _cache["nc"] = _build_nc()
    return _cache["nc"]


def _make_in_maps(x):
    import ml_dtypes

    bf16 = ml_dtypes.bfloat16
    idx = np.arange(1, L + 1, dtype=np.float64)
    rrow = (1.0 / idx).astype(bf16).reshape(1, L)
    in_maps = []
    shards = []
    for c in range(NCORES):
        b, dh = c // 2, c % 2
        shards.append((b, dh))
        xT = np.ascontiguousarray(x[b, :, dh * P : (dh + 1) * P].T)
        in_maps.append({"xT": xT, "rrow": rrow})
    return in_maps, shards


def kernel(x, q):
    from concourse.bass_utils import run_bass_kernel_spmd

    x = np.asarray(x)
    assert x.shape == (B, L, D) and x.dtype == np.float32

    nc = _get_nc()
    in_maps, shards = _make_in_maps(x)
    results = run_bass_kernel_spmd(nc, in_maps, list(range(NCORES))).results

    out = np.empty((B, L, D), dtype=np.float32)
    for c, (b, dh) in enumerate(shards):
        out[b, :, dh * P : (dh + 1) * P] = results[c]["out"].T.astype(np.float32)
    return out

